# revision 1
# baseline (speedup 1.0000x reference)
"""Trainium2 Bass kernel for AttentionFixModel (topk_masking).

Computation (per (b,t) row):
  q_proj = queries @ W_in + b_in                       [B,T,D]
  scores = einsum('btd,btnd->btn', q_proj, patch)      [B,T,N]
  attn   = softmax(scores); top-16 hard mask; renorm
  out    = einsum('btn,btnd->btd', attn, patch) @ W_out + b_out

Sharding: data-parallel over batch. B=16 across 8 cores -> 2 batches
(32 rows) per core. Weights replicated.

Per-core structure (32 rows, 2 groups of 16):
  scores: PE broadcasts q_proj row into PSUM (identity-column one-hot),
          DVE multiplies vs patch (native [n,d] layout), ACT/DVE reduce.
  top-16: max8 + match_replace (exact, tie-compatible), exp fused with
          -max bias and denominator accumulation on ACT.
  weighted sum + projections on PE.
Emission is software-pipelined: group g+1's streaming scores are emitted
before group g's serial epilogue so engine queues never head-of-line
block.
"""
import os
import sys

for _p in ("/opt/trn_rl_repo", "/root/.axon_site/_ro/trn_rl_repo"):
    if _p not in sys.path and os.path.isdir(_p):
        sys.path.append(_p)

import numpy as np
import concourse.bass as bass
import concourse.bacc as bacc
import concourse.mybir as mybir
from concourse import masks
from concourse.tile import TileContext

F32 = mybir.dt.float32
F32R = mybir.dt.float32r
USE_F32R = os.environ.get('KF32R', '0') == '1'


def _r(ap):
    return ap.bitcast(F32R) if USE_F32R else ap
Alu = mybir.AluOpType
Act = mybir.ActivationFunctionType

B, T, N, D = 16, 16, 256, 384
QDIM = 384
TOPK = 16
EPS = 1e-8
NCORES = 8
BT = (B // NCORES) * T          # rows per core = 32
GROUP = 16                      # rows per pipeline group
NGROUPS = BT // GROUP
NH = N // 128                   # patch partition-halves (2)
ND = D // 128                   # d-dim 128-tiles (3)
NQ = QDIM // 128                # q-dim 128-tiles (3)


def build_kernel() -> bass.Bass:
    nc = bacc.Bacc("TRN2", target_bir_lowering=False)

    sm_d = nc.dram_tensor("smalls", [BT + 33, QDIM], F32, kind="ExternalInput")
    p_d = nc.dram_tensor("patch_features", [BT, N, D], F32, kind="ExternalInput")
    win_d = nc.dram_tensor("W_in", [QDIM, D], F32, kind="ExternalInput")
    wout_d = nc.dram_tensor("W_out", [D, QDIM], F32, kind="ExternalInput")
    out_d = nc.dram_tensor("out", [BT, QDIM], F32, kind="ExternalOutput")

    # DRAM view of patches: [p=128, bt, h, d]
    p_view = p_d[:].rearrange("bt (h p) d -> p bt h d", p=128)

    with TileContext(nc) as tc:
        with (
            tc.tile_pool(name="const", bufs=1) as cpool,
            tc.tile_pool(name="wgt", bufs=1) as wpool,
            tc.tile_pool(name="patch", bufs=9) as ppool,
            tc.tile_pool(name="scr", bufs=6) as spool,
            tc.tile_pool(name="rows", bufs=2) as rpool,
            tc.tile_pool(name="qb", bufs=3, space="PSUM") as qbpool,
            tc.tile_pool(name="ptr", bufs=2, space="PSUM") as trpool,
            tc.tile_pool(name="poc", bufs=2, space="PSUM") as ocpool,
            tc.tile_pool(name="pfin", bufs=1, space="PSUM") as finpool,
        ):
            # ---------- constants ----------
            ident = cpool.tile([128, 128], F32)
            masks.make_identity(nc, ident[:])
            ones_col = cpool.tile([128, 128], F32)
            nc.gpsimd.memset(ones_col[:], 1.0)

            # ---------- weights + smalls (queries/biases packed) ----------
            smalls = wpool.tile([BT + 33, QDIM], F32, tag="smalls")
            nc.sync.dma_start(smalls[:], sm_d[:])
            queries = smalls[:BT, :]
            b_in = smalls[32:33, :]
            b_out = smalls[64:65, :]
            w_in_t = wpool.tile([128, NQ, D], F32, tag="w_in_t")
            nc.sync.dma_start(w_in_t[:], win_d[:].rearrange("(j p) d -> p j d", p=128))
            w_out_t = wpool.tile([128, ND, QDIM], F32, tag="w_out_t")
            w_in = [w_in_t[:, j, :] for j in range(NQ)]
            w_out = [w_out_t[:, j, :] for j in range(ND)]

            chunk_rows = {}     # g -> list of (tile, row_in_tile)

            def emit_patch_dmas(g, sizes):
                r0 = g * GROUP
                rows, cb = [], 0
                for c, sz in enumerate(sizes):
                    pc = ppool.tile([128, sz, NH, D], F32, tag="patch",
                                    name=f"patch_g{g}c{c}")
                    nc.sync.dma_start(pc[:], p_view[:, r0 + cb:r0 + cb + sz])
                    rows += [(pc, i) for i in range(sz)]
                    cb += sz
                chunk_rows[g] = rows

            emit_patch_dmas(0, [2, 2, 4, 4, 4])
            nc.sync.dma_start(w_out_t[:],
                              wout_d[:].rearrange("(j p) d -> p j d", p=128))

            # ---------- q_proj = queries @ W_in + b_in ----------
            qT = []
            for j in range(NQ):
                tp = trpool.tile([128, BT], F32, tag="ptr")
                nc.tensor.transpose(tp[:], queries[:, 128 * j:128 * (j + 1)],
                                    ident[:BT, :BT])
                sb = wpool.tile([128, BT], F32, tag=f"qT{j}", name=f"qT{j}")
                nc.scalar.copy(sb[:], tp[:])
                qT.append(sb)
            qp_ps = finpool.tile([BT, D], F32, tag="pfin")
            for j in range(NQ):
                nc.tensor.matmul(qp_ps[:], qT[j][:], w_in[j],
                                 start=(j == 0), stop=False)
            nc.tensor.matmul(qp_ps[:], ones_col[32:33, :BT], b_in,
                             start=False, stop=True)
            qproj = wpool.tile([BT, D], F32, tag="qproj")
            nc.scalar.copy(qproj[:], qp_ps[:])

            # ---------- pipeline stages ----------
            state = {}          # g -> dict of tiles

            def emit_scores(g, rr):
                """Streaming part for rows rr of group g: bcast+mul+reduce."""
                r0 = g * GROUP
                if g not in state:
                    state[g] = {
                        "scol": rpool.tile([128, GROUP, NH], F32, tag="scol",
                                           name=f"scol_{g}")}
                scol = state[g]["scol"]
                rows = chunk_rows[g]
                for r in rr:
                    bt = r0 + r
                    qb = qbpool.tile([128, 512], F32, tag="qb")
                    nc.tensor.matmul(qb[:, :D],
                                     _r(ident[:BT, bt:bt + 1].broadcast_to((BT, 128))),
                                     _r(qproj[:]), start=True, stop=True)
                    pc, i = rows[r]
                    scratch = spool.tile([128, NH, D], F32, tag="scr")
                    nc.vector.tensor_mul(
                        scratch[:], pc[:, i, :, :],
                        qb[:, :D].unsqueeze(1).broadcast_to((128, NH, D)))
                    if r % 4 == 3:
                        nc.vector.tensor_reduce(out=scol[:, r, :],
                                                in_=scratch[:],
                                                axis=mybir.AxisListType.X,
                                                op=Alu.add)
                    else:
                        ascr = spool.tile([128, NH * D], F32, tag="ascr")
                        for h in range(NH):
                            nc.scalar.activation(
                                out=ascr[:, h * D:(h + 1) * D],
                                in_=scratch[:, h, :],
                                func=Act.Copy, bias=0.0, scale=1.0,
                                accum_out=scol[:, r, h:h + 1])

            def emit_epilogue(g, ra=0, rb=GROUP):
                """Serial part for rows [ra,rb): transpose, top-16, weighted, out."""
                r0 = g * GROUP
                NR = rb - ra
                scol = state[g]["scol"]
                rows = chunk_rows[g]

                srows = rpool.tile([NR, N], F32, tag="srows")
                for h in range(NH):
                    tp = trpool.tile([NR, 128], F32, tag="ptr")
                    nc.tensor.transpose(tp[:], scol[:, ra:rb, h], ident[:, :])
                    nc.scalar.copy(srows[:, 128 * h:128 * (h + 1)], tp[:])

                negm = rpool.tile([NR, 1], F32, tag="negm")
                nc.vector.tensor_reduce(out=negm[:], in_=srows[:],
                                        axis=mybir.AxisListType.X,
                                        op=Alu.max, negate=True)
                p_sb = rpool.tile([NR, N], F32, tag="p")
                zden = rpool.tile([NR, 1], F32, tag="z")
                nc.scalar.activation(out=p_sb[:], in_=srows[:], func=Act.Exp,
                                     bias=negm[:], scale=1.0, accum_out=zden[:])
                # zap top-16 of p to zero (two rounds of max8 + match_replace)
                m8 = rpool.tile([NR, 8], F32, tag="m8")
                w_sb = rpool.tile([NR, N], F32, tag="w")
                nc.vector.max(out=m8[:], in_=p_sb[:])
                nc.vector.match_replace(out=w_sb[:], in_to_replace=m8[:],
                                        in_values=p_sb[:], imm_value=0.0)
                nc.vector.max(out=m8[:], in_=w_sb[:])
                nc.vector.match_replace(out=w_sb[:], in_to_replace=m8[:],
                                        in_values=w_sb[:], imm_value=0.0)
                # pm = p - w (only top-16 survive); t = sum(pm)
                pm = rpool.tile([NR, N], F32, tag="pm")
                tsum = rpool.tile([NR, 1], F32, tag="t")
                nc.gpsimd.tensor_sub(pm[:], p_sb[:], w_sb[:])
                nc.vector.tensor_reduce(out=tsum[:], in_=pm[:],
                                        axis=mybir.AxisListType.X, op=Alu.add)
                den = rpool.tile([NR, 1], F32, tag="den")
                nc.vector.tensor_scalar(out=den[:], in0=zden[:], scalar1=EPS,
                                        scalar2=tsum[:], op0=Alu.mult,
                                        op1=Alu.add)
                winv = rpool.tile([NR, 1], F32, tag="winv")
                nc.vector.reciprocal(out=winv[:], in_=den[:])
                wf = rpool.tile([NR, N], F32, tag="wf")
                nc.gpsimd.tensor_scalar(out=wf[:], in0=pm[:], scalar1=winv[:],
                                        scalar2=None, op0=Alu.mult)

                wcol = []
                for h in range(NH):
                    tp = trpool.tile([128, NR], F32, tag="ptr")
                    nc.tensor.transpose(tp[:], wf[:, 128 * h:128 * (h + 1)],
                                        ident[:NR, :NR])
                    sb = rpool.tile([128, NR], F32, tag=f"wcol{h}",
                                    name=f"wcol{h}_{g}")
                    nc.scalar.copy(sb[:], tp[:])
                    wcol.append(sb)

                oc_ps = ocpool.tile([128, ND, NR], F32, tag="poc")
                for rr in range(NR):
                    pc, i = rows[ra + rr]
                    for j in range(ND):
                        for h in range(NH):
                            nc.tensor.matmul(
                                oc_ps[:, j, rr:rr + 1],
                                _r(pc[:, i, h, 128 * j:128 * (j + 1)]),
                                _r(wcol[h][:, rr:rr + 1]),
                                start=(h == 0), stop=(h == NH - 1))
                oc_sb = rpool.tile([128, ND, NR], F32, tag="oc")
                nc.scalar.copy(oc_sb[:], oc_ps[:])

                fin_ps = finpool.tile([NR, QDIM], F32, tag="pfin")
                for j in range(ND):
                    nc.tensor.matmul(fin_ps[:], oc_sb[:, j, :], w_out[j],
                                     start=(j == 0), stop=False)
                nc.tensor.matmul(fin_ps[:], ones_col[64:65, :NR], b_out,
                                 start=False, stop=True)
                fin_sb = rpool.tile([NR, QDIM], F32, tag="fin")
                nc.scalar.copy(fin_sb[:], fin_ps[:])
                nc.sync.dma_start(out_d[r0 + ra:r0 + rb, :], fin_sb[:])

            # ---------- software-pipelined emission ----------
            emit_scores(0, range(GROUP))
            for g in range(1, NGROUPS):
                emit_patch_dmas(g, [4] * (GROUP // 4))
                emit_scores(g, range(GROUP // 2))
                emit_epilogue(g - 1)
                emit_scores(g, range(GROUP // 2, GROUP))
            emit_epilogue(NGROUPS - 1)

    if not nc.is_finalized():
        nc.finalize()
    return nc


def make_in_maps(queries, patch, W_in, b_in, W_out, b_out):
    bpc = B // NCORES
    in_maps = []
    for c in range(NCORES):
        smalls = np.zeros((BT + 33, QDIM), np.float32)
        smalls[:BT] = queries[c * bpc:(c + 1) * bpc].reshape(BT, QDIM)
        smalls[32] = b_in[0]
        smalls[64] = b_out[0]
        in_maps.append({
            "smalls": smalls,
            "patch_features": np.ascontiguousarray(
                patch[c * bpc:(c + 1) * bpc].reshape(BT, N, D)),
            "W_in": W_in, "W_out": W_out,
        })
    return in_maps


_NC_CACHE = None


def kernel(**inputs) -> np.ndarray:
    global _NC_CACHE
    from concourse.bass_utils import run_bass_kernel_spmd

    queries = np.ascontiguousarray(inputs["queries"], dtype=np.float32)
    patch = np.ascontiguousarray(inputs["patch_features"], dtype=np.float32)
    W_in = np.ascontiguousarray(inputs["W_in"], dtype=np.float32)
    b_in = np.ascontiguousarray(inputs["b_in"], dtype=np.float32).reshape(1, D)
    W_out = np.ascontiguousarray(inputs["W_out"], dtype=np.float32)
    b_out = np.ascontiguousarray(inputs["b_out"], dtype=np.float32).reshape(1, QDIM)

    if _NC_CACHE is None:
        _NC_CACHE = build_kernel()
    nc = _NC_CACHE

    in_maps = make_in_maps(queries, patch, W_in, b_in, W_out, b_out)
    res = run_bass_kernel_spmd(nc, in_maps, core_ids=list(range(NCORES)))
    bpc = B // NCORES
    outs = [res.results[c]["out"].reshape(bpc, T, QDIM) for c in range(NCORES)]
    return np.concatenate(outs, axis=0)

